# revision 8
# baseline (speedup 1.0000x reference)
"""Trainium2 Bass kernel for nn_Decoder: bit-unpack 23x22-bit codes per batch
row, gather fp16 table rows by index, sign-flip about 0.5, scatter into a
[B, 2, 126, 128] fp32 output whose rows 19:67 carry data and the rest are 0.5.

Sharding: data-parallel over batch across 8 NeuronCores (1024 rows each); the
lookup table is replicated on every core.

Table repack (host-side, untimed): the original row is [2, 48, 8] fp16 =
1536B, but codes 0..13 only consume a 4-channel half ([2,48,0:4] for c<7,
[2,48,4:8] for 7<=c<14). We upload TN2[L, 768] fp16 whose row i is
[lo-half(i) | hi-half(i)]; narrow codes gather 768B at element_offset 0/384,
wide codes (14..22) gather the full 1536B row. Cuts gather HBM reads from
35328B to 24576B per batch row with a single 201MB table.

HW indirect gather consumes ONE offset per partition and fetches a contiguous
per-partition block (verified by probe; CoreSim's multi-offset generality does
NOT hold on HW) -> one DMA per code, 23 per group. Casting SWDGE stores were
tried and are ruinously slow at Q7 desc-gen (~22us per 6MB store) - stores
must stay on HWDGE.

Ring plan (one HWDGE ring tops out ~310GB/s alone; two rings + SWDGE sustain
430+): x loads go first on the SP ring (ring FIFO drains in issue order, so
they must beat the fills in). Each group's output store is split into two
half-stores (p=0 plane on SP, p=1 on ACT) so both rings carry ~67MB and the
od buffer recycles at twice the single-store rate. Constant fills alternate
rings and are interleaved ~3 groups ahead of the half-stores so a store's
semaphore wait never leaves its ring without queued work. Gathers ride SWDGE
queues 0/1; Pool does nothing else.

Self-contained: hardcodes all shapes; no imports from the problem directory.
"""

import numpy as np

import concourse.bacc as bacc
import concourse.bass as bass
import concourse.mybir as mybir
import concourse.tile as tile

# Problem constants (hardcoded per contract)
BATCH = 8192
XCOLS = 512          # 6 + 23*22
NCODE = 23
NBITS = 22
L = 131072           # table rows
ROW = 768            # fp16 elements per repacked row [lo 384 | hi 384]
HROW = 384
NCORES = 8
BC = BATCH // NCORES  # 1024 rows per core
P = 128
GROUPS = BC // P      # 8 groups of 128 batch rows

# Output geometry: out[b] is [2, 126, 128] fp32 = [p, r, c].
# Data rows are r in [19, 67); flattened per-b layout [32256]:
#   [0:2432) = 0.5 | [2432:8576) p0 data | [8576:18560) = 0.5 |
#   [18560:24704) p1 data | [24704:32256) = 0.5
F_ROW = 126 * 128     # 16128 per p
D_LO = 19 * 128       # 2432
D_HI = 67 * 128       # 8576
GAP_MID = (126 - 67 + 19) * 128   # 9984
GAP_HI = (126 - 67) * 128         # 7552
C05W = GAP_HI // 2    # 3776: fill-source tile width
DW = D_HI - D_LO      # 6144: data span per p plane

f16 = mybir.dt.float16
f32 = mybir.dt.float32
i32 = mybir.dt.int32


N_SWDGE_QUEUES = 2


def build_module():
    nc = bacc.Bacc(
        "TRN2", target_bir_lowering=False, debug=False,
        num_swdge_queues=N_SWDGE_QUEUES,
    )
    x_t = nc.dram_tensor("x", [BC, XCOLS], i32, kind="ExternalInput")
    tn_t = nc.dram_tensor("table", [L, ROW], f16, kind="ExternalInput")
    w_t = nc.dram_tensor("w", [P, NCODE * NBITS], f32, kind="ExternalInput")
    out_t = nc.dram_tensor("out", [BC, 2, 126, 128], f32, kind="ExternalOutput")

    outf = out_t[:].rearrange("b p r c -> b (p r c)")    # [BC, 32256]

    with tile.TileContext(nc) as tc:
        with (
            tc.tile_pool(name="const", bufs=1) as cpool,
            tc.tile_pool(name="xl", bufs=GROUPS) as xlpool,
            tc.tile_pool(name="xp", bufs=2) as xpool,
            tc.tile_pool(name="sm", bufs=GROUPS) as spool,
            tc.tile_pool(name="gn", bufs=28) as gnpool,
            tc.tile_pool(name="gw", bufs=20) as gwpool,
            tc.tile_pool(name="op", bufs=2) as opool,
        ):
            w_tile = cpool.tile([P, NCODE * NBITS], f32)
            nc.sync.dma_start(w_tile[:], w_t[:])
            c05 = cpool.tile([P, C05W], f32)
            nc.vector.memset(c05[:], 0.5)

            # x loads first: they must hit the SP ring before the fills
            # clog its FIFO (ring drains in issue order).
            x_tiles = []
            for g in range(GROUPS):
                x_tile = xlpool.tile([P, XCOLS], i32)
                nc.sync.dma_start(x_tile[:], x_t[g * P : (g + 1) * P, :])
                x_tiles.append(x_tile)

            # Constant fills: a work-list popped in emission order. Most
            # drain in the ring-idle ramp (prelude) and after the last
            # store (tail); exactly one sits between consecutive stores so
            # a store's semaphore wait never parks a long fill train.
            fill_regions = []
            for g in range(GROUPS):
                for lo, hi in (
                    (0, D_LO),
                    (D_HI, D_HI + C05W),
                    (D_HI + C05W, D_HI + GAP_HI),
                    (D_HI + GAP_HI, D_HI + GAP_MID),
                    (F_ROW + D_HI, F_ROW + D_HI + C05W),
                    (F_ROW + D_HI + C05W, 2 * F_ROW),
                ):
                    fill_regions.append((lo, hi, g))
            fill_regions.reverse()  # pop() emits in natural order
            fill_engs = [nc.sync, nc.scalar]
            nfill = 0

            def fill(n):
                nonlocal nfill
                for _ in range(n):
                    if not fill_regions:
                        return
                    lo, hi, g = fill_regions.pop()
                    b0 = g * P
                    fill_engs[nfill % 2].dma_start(
                        out=outf[b0 : b0 + P, lo:hi], in_=c05[:, 0 : hi - lo]
                    )
                    nfill += 1

            # Decode all idx/sign tiles up-front so the gather stream is
            # never gated on the Vector chain mid-flight.
            idxs, tts, sgs = [], [], []
            for g in range(GROUPS):
                xf = xpool.tile([P, XCOLS], f32)
                nc.vector.tensor_copy(out=xf[:], in_=x_tiles[g][:])
                prod = xpool.tile([P, NCODE * NBITS], f32)
                nc.vector.tensor_tensor(
                    out=prod[:], in0=xf[:, 6:], in1=w_tile[:],
                    op=mybir.AluOpType.mult,
                )
                codes = xpool.tile([P, NCODE], f32, tag="codes")
                nc.vector.tensor_reduce(
                    out=codes[:],
                    in_=prod[:].rearrange("n (c a) -> n c a", a=NBITS),
                    axis=mybir.AxisListType.X,
                    op=mybir.AluOpType.add,
                )
                codesi = xpool.tile([P, NCODE], i32, tag="codesi")
                nc.vector.tensor_copy(out=codesi[:], in_=codes[:])
                idx = spool.tile([P, NCODE], i32, tag="idx")
                nc.vector.tensor_scalar(
                    out=idx[:], in0=codesi[:],
                    scalar1=L - 1, scalar2=None,
                    op0=mybir.AluOpType.bitwise_and,
                )
                # tt = 1.0 where codes > L else 0.0 ; sign = 1 - 2*tt
                tt = spool.tile([P, NCODE], f32, tag="tt")
                nc.vector.tensor_scalar(
                    out=tt[:], in0=codes[:],
                    scalar1=float(L), scalar2=None,
                    op0=mybir.AluOpType.is_gt,
                )
                sg = spool.tile([P, NCODE], f32, tag="sg")
                nc.vector.tensor_scalar(
                    out=sg[:], in0=tt[:],
                    scalar1=-2.0, scalar2=1.0,
                    op0=mybir.AluOpType.mult, op1=mybir.AluOpType.add,
                )
                idxs.append(idx); tts.append(tt); sgs.append(sg)

            # Prime both rings with enough fills to cover the pipeline ramp.
            fill(8)

            # Gather + permute + store stream.
            def emit_val(out_ap, in_ap, sg, tt, c):
                # val = sign*g + tt  (== 0.5 + sign*(g-0.5))
                nc.vector.tensor_scalar(
                    out=out_ap, in0=in_ap,
                    scalar1=sg[:, c : c + 1],
                    scalar2=tt[:, c : c + 1],
                    op0=mybir.AluOpType.mult,
                    op1=mybir.AluOpType.add,
                )

            for g in range(GROUPS):
                b0 = g * P
                idx, tt, sg = idxs[g], tts[g], sgs[g]
                od = opool.tile([P, 2 * DW], f32)
                od4 = od[:].rearrange("n (p k c) -> n p k c", p=2, k=48)
                for c in range(NCODE):
                    wide = c >= 14
                    gc = (gwpool if wide else gnpool).tile(
                        [P, ROW if wide else HROW], f16
                    )
                    gi = nc.gpsimd.indirect_dma_start(
                        out=gc[:],
                        out_offset=None,
                        in_=tn_t[:],
                        in_offset=bass.IndirectOffsetOnAxis(
                            ap=idx[:, c : c + 1], axis=0
                        ),
                        element_offset=HROW if 7 <= c < 14 else 0,
                    )
                    if c % 2:
                        gi.ins.queue = "qPoolDynamic1"
                    if wide:
                        col0 = (c - 7) * 8
                        glo = gc[:, 0:HROW].rearrange(
                            "n (p k c) -> n p k c", p=2, k=48
                        )
                        ghi = gc[:, HROW:ROW].rearrange(
                            "n (p k c) -> n p k c", p=2, k=48
                        )
                        emit_val(od4[:, :, :, col0 : col0 + 4], glo[:], sg, tt, c)
                        emit_val(od4[:, :, :, col0 + 4 : col0 + 8], ghi[:], sg, tt, c)
                    else:
                        col0 = c * 8 if c < 7 else (c - 7) * 8 + 4
                        gv = gc[:].rearrange("n (p k c) -> n p k c", p=2, k=48)
                        emit_val(od4[:, :, :, col0 : col0 + 4], gv[:], sg, tt, c)
                # Half-stores: p0 plane on SP ring, p1 plane on ACT ring,
                # then one small fill per ring behind each.
                nc.sync.dma_start(
                    out=outf[b0 : b0 + P, D_LO:D_HI], in_=od[:, 0:DW]
                )
                nc.scalar.dma_start(
                    out=outf[b0 : b0 + P, F_ROW + D_LO : F_ROW + D_HI],
                    in_=od[:, DW : 2 * DW],
                )
                fill(2)
            # Remaining fills drain freely after the last store is pushed.
            fill(len(fill_regions))
    nc.compile()
    return nc


def make_weights():
    w = np.tile((2.0 ** np.arange(NBITS)).astype(np.float32), NCODE)
    return np.broadcast_to(w, (P, NCODE * NBITS)).copy()


def make_tn(table):
    t = np.asarray(table).reshape(L, 2, 48, 8)
    tn = np.empty((L, ROW), dtype=np.float16)
    tn[:, :HROW] = t[:, :, :, 0:4].reshape(L, HROW)
    tn[:, HROW:] = t[:, :, :, 4:8].reshape(L, HROW)
    return tn


def make_in_maps(x, table):
    tn = make_tn(table)
    w = make_weights()
    return [
        {
            "x": np.ascontiguousarray(x[i * BC : (i + 1) * BC]),
            "table": tn,
            "w": w,
        }
        for i in range(NCORES)
    ]


_NC_CACHE = None


def _get_module():
    global _NC_CACHE
    if _NC_CACHE is None:
        _NC_CACHE = build_module()
    return _NC_CACHE


def kernel(x: np.ndarray, table: np.ndarray) -> np.ndarray:
    from concourse.bass_utils import run_bass_kernel_spmd

    x = np.asarray(x)
    table = np.asarray(table)
    assert x.shape == (BATCH, XCOLS) and table.shape == (L, 2, 48, 8)
    nc = _get_module()
    res = run_bass_kernel_spmd(nc, make_in_maps(x, table), core_ids=list(range(NCORES)))
    return np.concatenate([res.results[i]["out"] for i in range(NCORES)], axis=0)


# revision 11
# speedup vs baseline: 1.0003x; 1.0003x over previous
"""Trainium2 Bass kernel for nn_Decoder: bit-unpack 23x22-bit codes per batch
row, gather fp16 table rows by index, sign-flip about 0.5, scatter into a
[B, 2, 126, 128] fp32 output whose rows 19:67 carry data and the rest are 0.5.

Sharding: data-parallel over batch across 8 NeuronCores (1024 rows each); the
lookup table is replicated on every core.

Table repack (host-side, untimed): the original row is [2, 48, 8] fp16 =
1536B, but codes 0..13 only consume a 4-channel half ([2,48,0:4] for c<7,
[2,48,4:8] for 7<=c<14). We upload TN2[L, 768] fp16 whose row i is
[lo-half(i) | hi-half(i)]; narrow codes gather 768B at element_offset 0/384,
wide codes (14..22) gather the full 1536B row. Cuts gather HBM reads from
35328B to 24576B per batch row with a single 201MB table.

HW indirect gather consumes ONE offset per partition and fetches a contiguous
per-partition block (verified by probe; CoreSim's multi-offset generality
does NOT hold on HW) -> one DMA per code, 23 per group. Casting SWDGE stores
were tried and are ruinously slow at Q7 desc-gen - stores stay on HWDGE.

Scheduling model learned from traces: the SBUF AXI fabric (~435GB/s) binds -
every output byte is an SBUF read, every gather byte an SBUF write. The
output-staging recycle is the governor that keeps us above that floor, so the
od tile is split per output plane with depths 3 (p0) / 2 (p1): store latency
is amortized instead of sitting on every group's critical path. Stores get a
dedicated SP ring (a store behind queued fills inherits their FIFO drain
delay); fills stream on the ACT ring, except a tail batch emitted on SP after
its last store so both rings stay busy to the end. Gathers ride SWDGE q0/q1.

Self-contained: hardcodes all shapes; no imports from the problem directory.
"""

import numpy as np

import concourse.bacc as bacc
import concourse.bass as bass
import concourse.mybir as mybir
import concourse.tile as tile

# Problem constants (hardcoded per contract)
BATCH = 8192
XCOLS = 512          # 6 + 23*22
NCODE = 23
NBITS = 22
L = 131072           # table rows
ROW = 768            # fp16 elements per repacked row [lo 384 | hi 384]
HROW = 384
QROW = 192           # one (plane) quarter of a repacked row
NCORES = 8
BC = BATCH // NCORES  # 1024 rows per core
P = 128
GROUPS = BC // P      # 8 groups of 128 batch rows

# Output geometry: out[b] is [2, 126, 128] fp32 = [p, r, c].
# Data rows are r in [19, 67); flattened per-b layout [32256]:
#   [0:2432) = 0.5 | [2432:8576) p0 data | [8576:18560) = 0.5 |
#   [18560:24704) p1 data | [24704:32256) = 0.5
F_ROW = 126 * 128     # 16128 per p
D_LO = 19 * 128       # 2432
D_HI = 67 * 128       # 8576
GAP_MID = (126 - 67 + 19) * 128   # 9984
GAP_HI = (126 - 67) * 128         # 7552
C05W = GAP_HI // 2    # 3776: fill-source tile width
DW = D_HI - D_LO      # 6144: data span per p plane

# Fills kept on the ACT ring vs deferred to the SP ring's post-store tail
# (balances ring byte loads: SP 50.3MB stores + ~21MB fills vs ACT ~63MB).
N_TAIL_SP_FILLS = 12

f16 = mybir.dt.float16
f32 = mybir.dt.float32
i32 = mybir.dt.int32


N_SWDGE_QUEUES = 2


def build_module():
    nc = bacc.Bacc(
        "TRN2", target_bir_lowering=False, debug=False,
        num_swdge_queues=N_SWDGE_QUEUES,
    )
    x_t = nc.dram_tensor("x", [BC, XCOLS], i32, kind="ExternalInput")
    tn_t = nc.dram_tensor("table", [L, ROW], f16, kind="ExternalInput")
    w_t = nc.dram_tensor("w", [P, NCODE * NBITS], f32, kind="ExternalInput")
    out_t = nc.dram_tensor("out", [BC, 2, 126, 128], f32, kind="ExternalOutput")

    outf = out_t[:].rearrange("b p r c -> b (p r c)")    # [BC, 32256]

    with tile.TileContext(nc) as tc:
        with (
            tc.tile_pool(name="const", bufs=1) as cpool,
            tc.tile_pool(name="xl", bufs=GROUPS) as xlpool,
            tc.tile_pool(name="xp", bufs=2) as xpool,
            tc.tile_pool(name="sm", bufs=GROUPS) as spool,
            tc.tile_pool(name="gn", bufs=16) as gnpool,
            tc.tile_pool(name="gw", bufs=12) as gwpool,
            tc.tile_pool(name="o0", bufs=3) as o0pool,
            tc.tile_pool(name="o1", bufs=2) as o1pool,
        ):
            w_tile = cpool.tile([P, NCODE * NBITS], f32)
            nc.sync.dma_start(w_tile[:], w_t[:])
            c05 = cpool.tile([P, C05W], f32)
            nc.vector.memset(c05[:], 0.5)

            # x loads first: they must hit the SP ring before anything else
            # queues on it (ring drains in issue order).
            x_tiles = []
            for g in range(GROUPS):
                x_tile = xlpool.tile([P, XCOLS], i32)
                nc.sync.dma_start(x_tile[:], x_t[g * P : (g + 1) * P, :])
                x_tiles.append(x_tile)

            # Fill work-list; most stream on ACT, the last N_TAIL_SP_FILLS
            # are emitted on SP after its final store.
            fill_regions = []
            for g in range(GROUPS):
                for lo, hi in (
                    (0, D_LO),
                    (D_HI, D_HI + C05W),
                    (D_HI + C05W, D_HI + GAP_HI),
                    (D_HI + GAP_HI, D_HI + GAP_MID),
                    (F_ROW + D_HI, F_ROW + D_HI + C05W),
                    (F_ROW + D_HI + C05W, 2 * F_ROW),
                ):
                    fill_regions.append((lo, hi, g))

            def fill(eng, regions):
                for lo, hi, g in regions:
                    b0 = g * P
                    eng.dma_start(
                        out=outf[b0 : b0 + P, lo:hi], in_=c05[:, 0 : hi - lo]
                    )

            fill(nc.scalar, fill_regions[:-N_TAIL_SP_FILLS])

            # Decode all idx/sign tiles up-front so the gather stream is
            # never gated on the Vector chain mid-flight.
            idxs, tts, sgs = [], [], []
            for g in range(GROUPS):
                xf = xpool.tile([P, XCOLS], f32)
                nc.vector.tensor_copy(out=xf[:], in_=x_tiles[g][:])
                prod = xpool.tile([P, NCODE * NBITS], f32)
                nc.vector.tensor_tensor(
                    out=prod[:], in0=xf[:, 6:], in1=w_tile[:],
                    op=mybir.AluOpType.mult,
                )
                codes = xpool.tile([P, NCODE], f32, tag="codes")
                nc.vector.tensor_reduce(
                    out=codes[:],
                    in_=prod[:].rearrange("n (c a) -> n c a", a=NBITS),
                    axis=mybir.AxisListType.X,
                    op=mybir.AluOpType.add,
                )
                codesi = xpool.tile([P, NCODE], i32, tag="codesi")
                nc.vector.tensor_copy(out=codesi[:], in_=codes[:])
                idx = spool.tile([P, NCODE], i32, tag="idx")
                nc.vector.tensor_scalar(
                    out=idx[:], in0=codesi[:],
                    scalar1=L - 1, scalar2=None,
                    op0=mybir.AluOpType.bitwise_and,
                )
                # tt = 1.0 where codes > L else 0.0 ; sign = 1 - 2*tt
                tt = spool.tile([P, NCODE], f32, tag="tt")
                nc.vector.tensor_scalar(
                    out=tt[:], in0=codes[:],
                    scalar1=float(L), scalar2=None,
                    op0=mybir.AluOpType.is_gt,
                )
                sg = spool.tile([P, NCODE], f32, tag="sg")
                nc.vector.tensor_scalar(
                    out=sg[:], in0=tt[:],
                    scalar1=-2.0, scalar2=1.0,
                    op0=mybir.AluOpType.mult, op1=mybir.AluOpType.add,
                )
                idxs.append(idx); tts.append(tt); sgs.append(sg)

            # Gather + permute + store stream.
            def emit_val(out_ap, in_ap, sg, tt, c):
                # val = sign*g + tt  (== 0.5 + sign*(g-0.5))
                nc.vector.tensor_scalar(
                    out=out_ap, in0=in_ap,
                    scalar1=sg[:, c : c + 1],
                    scalar2=tt[:, c : c + 1],
                    op0=mybir.AluOpType.mult,
                    op1=mybir.AluOpType.add,
                )

            for g in range(GROUPS):
                b0 = g * P
                idx, tt, sg = idxs[g], tts[g], sgs[g]
                od = [
                    o0pool.tile([P, DW], f32, name="od0", tag="od0"),
                    o1pool.tile([P, DW], f32, name="od1", tag="od1"),
                ]
                odv = [
                    t[:].rearrange("n (k c) -> n k c", c=128) for t in od
                ]
                for c in range(NCODE):
                    wide = c >= 14
                    gc = (gwpool if wide else gnpool).tile(
                        [P, ROW if wide else HROW], f16
                    )
                    gi = nc.gpsimd.indirect_dma_start(
                        out=gc[:],
                        out_offset=None,
                        in_=tn_t[:],
                        in_offset=bass.IndirectOffsetOnAxis(
                            ap=idx[:, c : c + 1], axis=0
                        ),
                        element_offset=HROW if 7 <= c < 14 else 0,
                    )
                    if c % 2:
                        gi.ins.queue = "qPoolDynamic1"
                    # per (code, half, plane) compute: gc is [(half,) p, k, 4]
                    nhalf = 2 if wide else 1
                    base_col = (c - 7) * 8 if c >= 7 else c * 8
                    for h in range(nhalf):
                        col0 = base_col + (
                            4 * h if wide else (0 if c < 7 else 4)
                        )
                        for p in range(2):
                            q0 = (2 * h + p) * QROW
                            gq = gc[:, q0 : q0 + QROW].rearrange(
                                "n (k c) -> n k c", c=4
                            )
                            emit_val(
                                odv[p][:, :, col0 : col0 + 4], gq[:], sg, tt, c
                            )
                # Both plane-stores on the dedicated SP ring.
                for p in range(2):
                    nc.sync.dma_start(
                        out=outf[b0 : b0 + P, p * F_ROW + D_LO : p * F_ROW + D_HI],
                        in_=od[p][:],
                    )
            # SP ring tail fills: drain freely after its last store.
            fill(nc.sync, fill_regions[-N_TAIL_SP_FILLS:])
    nc.compile()
    return nc


def make_weights():
    w = np.tile((2.0 ** np.arange(NBITS)).astype(np.float32), NCODE)
    return np.broadcast_to(w, (P, NCODE * NBITS)).copy()


def make_tn(table):
    t = np.asarray(table).reshape(L, 2, 48, 8)
    tn = np.empty((L, ROW), dtype=np.float16)
    tn[:, :HROW] = t[:, :, :, 0:4].reshape(L, HROW)
    tn[:, HROW:] = t[:, :, :, 4:8].reshape(L, HROW)
    return tn


def make_in_maps(x, table):
    tn = make_tn(table)
    w = make_weights()
    return [
        {
            "x": np.ascontiguousarray(x[i * BC : (i + 1) * BC]),
            "table": tn,
            "w": w,
        }
        for i in range(NCORES)
    ]


_NC_CACHE = None


def _get_module():
    global _NC_CACHE
    if _NC_CACHE is None:
        _NC_CACHE = build_module()
    return _NC_CACHE


def kernel(x: np.ndarray, table: np.ndarray) -> np.ndarray:
    from concourse.bass_utils import run_bass_kernel_spmd

    x = np.asarray(x)
    table = np.asarray(table)
    assert x.shape == (BATCH, XCOLS) and table.shape == (L, 2, 48, 8)
    nc = _get_module()
    res = run_bass_kernel_spmd(nc, make_in_maps(x, table), core_ids=list(range(NCORES)))
    return np.concatenate([res.results[i]["out"] for i in range(NCORES)], axis=0)


# revision 12
# speedup vs baseline: 1.1500x; 1.1497x over previous
"""Trainium2 Bass kernel for nn_Decoder: bit-unpack 23x22-bit codes per batch
row, gather fp16 table rows by index, sign-flip about 0.5, scatter into a
[B, 2, 126, 128] fp32 output whose rows 19:67 carry data and the rest are 0.5.

Sharding: data-parallel over batch across 8 NeuronCores (1024 rows each); the
lookup table is replicated on every core.

Table repack (host-side, untimed): the original row is [2, 48, 8] fp16 =
1536B, but codes 0..13 only consume a 4-channel half ([2,48,0:4] for c<7,
[2,48,4:8] for 7<=c<14). We upload TN2[L, 768] fp16 whose row i is
[lo-half(i) | hi-half(i)]; narrow codes gather 768B at element_offset 0/384,
wide codes (14..22) gather the full 1536B row. Cuts gather HBM reads from
35328B to 24576B per batch row with a single 201MB table.

HW indirect gather consumes ONE offset per partition and fetches a contiguous
per-partition block (probe-verified; CoreSim's multi-offset generality does
NOT hold on HW) -> one DMA per code, 23 per group. Casting SWDGE stores are
ruinously slow at Q7 desc-gen - stores stay on HWDGE.

Scheduling (distilled from 6 traced variants): the output-store stream is
the governor - 50MB through one ring's round-robin share (~140GB/s when
fills/gathers are active) is ~360us of serial store time. So each group's
store is split across BOTH rings (p0 plane on SP, p1 on ACT), the od staging
tile is 3-deep so a ~40us store turnaround stays off the per-group critical
path, and exactly two small fills per ring follow each half-store - they
drain inside the od-wait gaps instead of parking in front of later stores
(fills-ahead-of-stores measurably adds their drain time to every od
recycle). Remaining fills run out on both rings after the last store, where
they saturate the write path with nothing to block. DVE keeps v2's 32
ops/group shape (one op spans both output planes; doubling op count for a
per-plane split made DVE the bottleneck).

Self-contained: hardcodes all shapes; no imports from the problem directory.
"""

import numpy as np

import concourse.bacc as bacc
import concourse.bass as bass
import concourse.mybir as mybir
import concourse.tile as tile

# Problem constants (hardcoded per contract)
BATCH = 8192
XCOLS = 512          # 6 + 23*22
NCODE = 23
NBITS = 22
L = 131072           # table rows
ROW = 768            # fp16 elements per repacked row [lo 384 | hi 384]
HROW = 384
NCORES = 8
BC = BATCH // NCORES  # 1024 rows per core
P = 128
GROUPS = BC // P      # 8 groups of 128 batch rows

# Output geometry: out[b] is [2, 126, 128] fp32 = [p, r, c].
# Data rows are r in [19, 67); flattened per-b layout [32256]:
#   [0:2432) = 0.5 | [2432:8576) p0 data | [8576:18560) = 0.5 |
#   [18560:24704) p1 data | [24704:32256) = 0.5
F_ROW = 126 * 128     # 16128 per p
D_LO = 19 * 128       # 2432
D_HI = 67 * 128       # 8576
GAP_MID = (126 - 67 + 19) * 128   # 9984
GAP_HI = (126 - 67) * 128         # 7552
C05W = 2432           # fill-source tile width
DW = D_HI - D_LO      # 6144: data span per p plane

f16 = mybir.dt.float16
f32 = mybir.dt.float32
i32 = mybir.dt.int32


N_SWDGE_QUEUES = 2


def _fill_regions():
    """Per-group constant-0.5 spans, each <= C05W wide."""
    spans = []

    def span(lo, hi):
        while hi - lo > C05W:
            spans.append((lo, lo + C05W))
            lo += C05W
        spans.append((lo, hi))

    span(0, D_LO)
    span(D_HI, D_HI + GAP_MID)
    span(F_ROW + D_HI, 2 * F_ROW)
    return spans


def build_module():
    nc = bacc.Bacc(
        "TRN2", target_bir_lowering=False, debug=False,
        num_swdge_queues=N_SWDGE_QUEUES,
    )
    x_t = nc.dram_tensor("x", [BC, XCOLS], i32, kind="ExternalInput")
    tn_t = nc.dram_tensor("table", [L, ROW], f16, kind="ExternalInput")
    w_t = nc.dram_tensor("w", [P, NCODE * NBITS], f32, kind="ExternalInput")
    out_t = nc.dram_tensor("out", [BC, 2, 126, 128], f32, kind="ExternalOutput")

    outf = out_t[:].rearrange("b p r c -> b (p r c)")    # [BC, 32256]

    regions = [(lo, hi, g) for g in range(GROUPS) for lo, hi in _fill_regions()]
    fill_i = 0
    nfill = 0

    with tile.TileContext(nc) as tc:
        with (
            tc.tile_pool(name="const", bufs=1) as cpool,
            tc.tile_pool(name="xp", bufs=2) as xpool,
            tc.tile_pool(name="sm", bufs=GROUPS) as spool,
            tc.tile_pool(name="gn", bufs=16) as gnpool,
            tc.tile_pool(name="gw", bufs=12) as gwpool,
            tc.tile_pool(name="op", bufs=3) as opool,
        ):
            w_tile = cpool.tile([P, NCODE * NBITS], f32)
            nc.sync.dma_start(w_tile[:], w_t[:])
            c05 = cpool.tile([P, C05W], f32)
            nc.vector.memset(c05[:], 0.5)

            fill_engs = [nc.sync, nc.scalar]

            def fill(n):
                nonlocal fill_i, nfill
                for _ in range(n):
                    if fill_i >= len(regions):
                        return
                    lo, hi, g = regions[fill_i]
                    fill_i += 1
                    b0 = g * P
                    fill_engs[nfill % 2].dma_start(
                        out=outf[b0 : b0 + P, lo:hi], in_=c05[:, 0 : hi - lo]
                    )
                    nfill += 1

            # Decode all idx/sign tiles up-front so the gather stream is
            # never gated on the Vector chain mid-flight. x loads ride the
            # SP ring ahead of every store/fill.
            idxs, tts, sgs = [], [], []
            for g in range(GROUPS):
                x_tile = xpool.tile([P, XCOLS], i32)
                nc.sync.dma_start(x_tile[:], x_t[g * P : (g + 1) * P, :])
                xf = xpool.tile([P, XCOLS], f32)
                nc.vector.tensor_copy(out=xf[:], in_=x_tile[:])
                prod = xpool.tile([P, NCODE * NBITS], f32)
                nc.vector.tensor_tensor(
                    out=prod[:], in0=xf[:, 6:], in1=w_tile[:],
                    op=mybir.AluOpType.mult,
                )
                codes = xpool.tile([P, NCODE], f32, tag="codes")
                nc.vector.tensor_reduce(
                    out=codes[:],
                    in_=prod[:].rearrange("n (c a) -> n c a", a=NBITS),
                    axis=mybir.AxisListType.X,
                    op=mybir.AluOpType.add,
                )
                codesi = xpool.tile([P, NCODE], i32, tag="codesi")
                nc.vector.tensor_copy(out=codesi[:], in_=codes[:])
                idx = spool.tile([P, NCODE], i32, tag="idx")
                nc.vector.tensor_scalar(
                    out=idx[:], in0=codesi[:],
                    scalar1=L - 1, scalar2=None,
                    op0=mybir.AluOpType.bitwise_and,
                )
                # tt = 1.0 where codes > L else 0.0 ; sign = 1 - 2*tt
                tt = spool.tile([P, NCODE], f32, tag="tt")
                nc.vector.tensor_scalar(
                    out=tt[:], in0=codes[:],
                    scalar1=float(L), scalar2=None,
                    op0=mybir.AluOpType.is_gt,
                )
                sg = spool.tile([P, NCODE], f32, tag="sg")
                nc.vector.tensor_scalar(
                    out=sg[:], in0=tt[:],
                    scalar1=-2.0, scalar2=1.0,
                    op0=mybir.AluOpType.mult, op1=mybir.AluOpType.add,
                )
                idxs.append(idx); tts.append(tt); sgs.append(sg)

            # Cover the pipeline ramp before the first store exists.
            fill(4)

            # Gather + permute + store stream.
            def emit_val(out_ap, in_ap, sg, tt, c):
                # val = sign*g + tt  (== 0.5 + sign*(g-0.5))
                nc.vector.tensor_scalar(
                    out=out_ap, in0=in_ap,
                    scalar1=sg[:, c : c + 1],
                    scalar2=tt[:, c : c + 1],
                    op0=mybir.AluOpType.mult,
                    op1=mybir.AluOpType.add,
                )

            for g in range(GROUPS):
                b0 = g * P
                idx, tt, sg = idxs[g], tts[g], sgs[g]
                od = opool.tile([P, 2 * DW], f32)
                od4 = od[:].rearrange("n (p k c) -> n p k c", p=2, k=48)
                for c in range(NCODE):
                    wide = c >= 14
                    gc = (gwpool if wide else gnpool).tile(
                        [P, ROW if wide else HROW], f16
                    )
                    gi = nc.gpsimd.indirect_dma_start(
                        out=gc[:],
                        out_offset=None,
                        in_=tn_t[:],
                        in_offset=bass.IndirectOffsetOnAxis(
                            ap=idx[:, c : c + 1], axis=0
                        ),
                        element_offset=HROW if 7 <= c < 14 else 0,
                    )
                    if c % 2:
                        gi.ins.queue = "qPoolDynamic1"
                    if wide:
                        col0 = (c - 7) * 8
                        glo = gc[:, 0:HROW].rearrange(
                            "n (p k c) -> n p k c", p=2, k=48
                        )
                        ghi = gc[:, HROW:ROW].rearrange(
                            "n (p k c) -> n p k c", p=2, k=48
                        )
                        emit_val(od4[:, :, :, col0 : col0 + 4], glo[:], sg, tt, c)
                        emit_val(od4[:, :, :, col0 + 4 : col0 + 8], ghi[:], sg, tt, c)
                    else:
                        col0 = c * 8 if c < 7 else (c - 7) * 8 + 4
                        gv = gc[:].rearrange("n (p k c) -> n p k c", p=2, k=48)
                        emit_val(od4[:, :, :, col0 : col0 + 4], gv[:], sg, tt, c)
                # Half-stores: p0 plane on SP, p1 on ACT; then two small
                # fills per ring drain inside the next od-wait gap.
                nc.sync.dma_start(
                    out=outf[b0 : b0 + P, D_LO:D_HI], in_=od[:, 0:DW]
                )
                nc.scalar.dma_start(
                    out=outf[b0 : b0 + P, F_ROW + D_LO : F_ROW + D_HI],
                    in_=od[:, DW : 2 * DW],
                )
                fill(4)
            # Remaining fills drain freely on both rings after the last
            # store is pushed.
            fill(len(regions))
    nc.compile()
    return nc


def make_weights():
    w = np.tile((2.0 ** np.arange(NBITS)).astype(np.float32), NCODE)
    return np.broadcast_to(w, (P, NCODE * NBITS)).copy()


def make_tn(table):
    t = np.asarray(table).reshape(L, 2, 48, 8)
    tn = np.empty((L, ROW), dtype=np.float16)
    tn[:, :HROW] = t[:, :, :, 0:4].reshape(L, HROW)
    tn[:, HROW:] = t[:, :, :, 4:8].reshape(L, HROW)
    return tn


def make_in_maps(x, table):
    tn = make_tn(table)
    w = make_weights()
    return [
        {
            "x": np.ascontiguousarray(x[i * BC : (i + 1) * BC]),
            "table": tn,
            "w": w,
        }
        for i in range(NCORES)
    ]


_NC_CACHE = None


def _get_module():
    global _NC_CACHE
    if _NC_CACHE is None:
        _NC_CACHE = build_module()
    return _NC_CACHE


def kernel(x: np.ndarray, table: np.ndarray) -> np.ndarray:
    from concourse.bass_utils import run_bass_kernel_spmd

    x = np.asarray(x)
    table = np.asarray(table)
    assert x.shape == (BATCH, XCOLS) and table.shape == (L, 2, 48, 8)
    nc = _get_module()
    res = run_bass_kernel_spmd(nc, make_in_maps(x, table), core_ids=list(range(NCORES)))
    return np.concatenate([res.results[i]["out"] for i in range(NCORES)], axis=0)


# revision 13
# speedup vs baseline: 1.2129x; 1.0547x over previous
"""Trainium2 Bass kernel for nn_Decoder: bit-unpack 23x22-bit codes per batch
row, gather fp16 table rows by index, sign-flip about 0.5, scatter into a
[B, 2, 126, 128] fp32 output whose rows 19:67 carry data and the rest are 0.5.

Sharding: data-parallel over batch across 8 NeuronCores (1024 rows each); the
lookup table is replicated on every core.

Table repack (host-side, untimed): the original row is [2, 48, 8] fp16 =
1536B, but codes 0..13 only consume a 4-channel half ([2,48,0:4] for c<7,
[2,48,4:8] for 7<=c<14). We upload TN2[L, 768] fp16 whose row i is
[lo-half(i) | hi-half(i)]; narrow codes gather 768B at element_offset 0/384,
wide codes (14..22) gather the full 1536B row. Cuts gather HBM reads from
35328B to 24576B per batch row with a single 201MB table.

HW indirect gather consumes ONE offset per partition and fetches a contiguous
per-partition block (probe-verified; CoreSim's multi-offset generality does
NOT hold on HW) -> one DMA per code, 23 per group; the ~1.1us/instruction
Pool desc-gen makes the gather stream span ~330us regardless of scheduling.

Ring schedule (best of 7 traced variants): ACT ring carries a deep backlog
of constant fills so it never idles; SP ring is dedicated to the whole
per-group stores (a store queued behind fills inherits their FIFO drain
delay straight onto the od-recycle critical path; splitting stores across
rings or pushing them through SWDGE both measured slower). The last few
fills are deferred: emitted on BOTH rings after the final store so the
write tail drains dual-ring instead of ACT-alone. Gathers ride SWDGE q0/q1
with 2 groups of buffering to keep the Pool desc-gen stream smooth.

Self-contained: hardcodes all shapes; no imports from the problem directory.
"""

import numpy as np

import concourse.bacc as bacc
import concourse.bass as bass
import concourse.mybir as mybir
import concourse.tile as tile

# Problem constants (hardcoded per contract)
BATCH = 8192
XCOLS = 512          # 6 + 23*22
NCODE = 23
NBITS = 22
L = 131072           # table rows
ROW = 768            # fp16 elements per repacked row [lo 384 | hi 384]
HROW = 384
NCORES = 8
BC = BATCH // NCORES  # 1024 rows per core
P = 128
GROUPS = BC // P      # 8 groups of 128 batch rows

# Output geometry: out[b] is [2, 126, 128] fp32 = [p, r, c].
# Data rows are r in [19, 67); flattened per-b layout [32256]:
#   [0:2432) = 0.5 | [2432:8576) p0 data | [8576:18560) = 0.5 |
#   [18560:24704) p1 data | [24704:32256) = 0.5
F_ROW = 126 * 128     # 16128 per p
D_LO = 19 * 128       # 2432
D_HI = 67 * 128       # 8576
GAP_MID = (126 - 67 + 19) * 128   # 9984
GAP_HI = (126 - 67) * 128         # 7552

# Fill spans per group (each <= GAP_HI wide, the c05 source width).
FILL_SPANS = (
    (0, D_LO),
    (D_HI, D_HI + GAP_HI),
    (D_HI + GAP_HI, D_HI + GAP_MID),
    (F_ROW + D_HI, 2 * F_ROW),
)
N_TAIL_FILLS = 10     # deferred to both rings after the last store

f16 = mybir.dt.float16
f32 = mybir.dt.float32
i32 = mybir.dt.int32


N_SWDGE_QUEUES = 2


def build_module():
    nc = bacc.Bacc(
        "TRN2", target_bir_lowering=False, debug=False,
        num_swdge_queues=N_SWDGE_QUEUES,
    )
    x_t = nc.dram_tensor("x", [BC, XCOLS], i32, kind="ExternalInput")
    tn_t = nc.dram_tensor("table", [L, ROW], f16, kind="ExternalInput")
    w_t = nc.dram_tensor("w", [P, NCODE * NBITS], f32, kind="ExternalInput")
    out_t = nc.dram_tensor("out", [BC, 2, 126, 128], f32, kind="ExternalOutput")

    outf = out_t[:].rearrange("b p r c -> b (p r c)")    # [BC, 32256]
    out3 = out_t[:].rearrange("b p r c -> b p (r c)")    # [BC, 2, 16128]

    regions = [(lo, hi, g) for g in range(GROUPS) for lo, hi in FILL_SPANS]

    with tile.TileContext(nc) as tc:
        with (
            tc.tile_pool(name="const", bufs=1) as cpool,
            tc.tile_pool(name="xp", bufs=2) as xpool,
            tc.tile_pool(name="sm", bufs=GROUPS) as spool,
            tc.tile_pool(name="gn", bufs=28) as gnpool,
            tc.tile_pool(name="gw", bufs=20) as gwpool,
            tc.tile_pool(name="op", bufs=2) as opool,
        ):
            w_tile = cpool.tile([P, NCODE * NBITS], f32)
            nc.sync.dma_start(w_tile[:], w_t[:])
            c05 = cpool.tile([P, GAP_HI], f32)
            nc.vector.memset(c05[:], 0.5)

            def fill(eng, rs):
                for lo, hi, g in rs:
                    b0 = g * P
                    eng.dma_start(
                        out=outf[b0 : b0 + P, lo:hi], in_=c05[:, 0 : hi - lo]
                    )

            # Deep fill backlog on the ACT ring: it never idles mid-run.
            fill(nc.scalar, regions[:-N_TAIL_FILLS])

            # Decode all idx/sign tiles up-front so the gather stream is
            # never gated on the Vector chain mid-flight. x loads ride the
            # SP ring ahead of every store.
            idxs, tts, sgs = [], [], []
            for g in range(GROUPS):
                b0 = g * P
                x_tile = xpool.tile([P, XCOLS], i32)
                nc.sync.dma_start(x_tile[:], x_t[b0 : b0 + P, :])
                xf = xpool.tile([P, XCOLS], f32)
                nc.vector.tensor_copy(out=xf[:], in_=x_tile[:])
                prod = xpool.tile([P, NCODE * NBITS], f32)
                nc.vector.tensor_tensor(
                    out=prod[:], in0=xf[:, 6:], in1=w_tile[:],
                    op=mybir.AluOpType.mult,
                )
                codes = xpool.tile([P, NCODE], f32, tag="codes")
                nc.vector.tensor_reduce(
                    out=codes[:],
                    in_=prod[:].rearrange("n (c a) -> n c a", a=NBITS),
                    axis=mybir.AxisListType.X,
                    op=mybir.AluOpType.add,
                )
                codesi = xpool.tile([P, NCODE], i32, tag="codesi")
                nc.vector.tensor_copy(out=codesi[:], in_=codes[:])
                idx = spool.tile([P, NCODE], i32, tag="idx")
                nc.vector.tensor_scalar(
                    out=idx[:], in0=codesi[:],
                    scalar1=L - 1, scalar2=None,
                    op0=mybir.AluOpType.bitwise_and,
                )
                # tt = 1.0 where codes > L else 0.0 ; sign = 1 - 2*tt
                tt = spool.tile([P, NCODE], f32, tag="tt")
                nc.vector.tensor_scalar(
                    out=tt[:], in0=codes[:],
                    scalar1=float(L), scalar2=None,
                    op0=mybir.AluOpType.is_gt,
                )
                sg = spool.tile([P, NCODE], f32, tag="sg")
                nc.vector.tensor_scalar(
                    out=sg[:], in0=tt[:],
                    scalar1=-2.0, scalar2=1.0,
                    op0=mybir.AluOpType.mult, op1=mybir.AluOpType.add,
                )
                idxs.append(idx); tts.append(tt); sgs.append(sg)

            # Gather + permute + store stream.
            def emit_val(out_ap, in_ap, sg, tt, c):
                # val = sign*g + tt  (== 0.5 + sign*(g-0.5))
                nc.vector.tensor_scalar(
                    out=out_ap, in0=in_ap,
                    scalar1=sg[:, c : c + 1],
                    scalar2=tt[:, c : c + 1],
                    op0=mybir.AluOpType.mult,
                    op1=mybir.AluOpType.add,
                )

            for g in range(GROUPS):
                b0 = g * P
                idx, tt, sg = idxs[g], tts[g], sgs[g]
                od = opool.tile([P, 2 * 48 * 128], f32)
                od4 = od[:].rearrange("n (p k c) -> n p k c", p=2, k=48)
                for c in range(NCODE):
                    wide = c >= 14
                    gc = (gwpool if wide else gnpool).tile(
                        [P, ROW if wide else HROW], f16
                    )
                    gi = nc.gpsimd.indirect_dma_start(
                        out=gc[:],
                        out_offset=None,
                        in_=tn_t[:],
                        in_offset=bass.IndirectOffsetOnAxis(
                            ap=idx[:, c : c + 1], axis=0
                        ),
                        element_offset=HROW if 7 <= c < 14 else 0,
                    )
                    if c % 2:
                        gi.ins.queue = "qPoolDynamic1"
                    if wide:
                        col0 = (c - 7) * 8
                        glo = gc[:, 0:HROW].rearrange(
                            "n (p k c) -> n p k c", p=2, k=48
                        )
                        ghi = gc[:, HROW:ROW].rearrange(
                            "n (p k c) -> n p k c", p=2, k=48
                        )
                        emit_val(od4[:, :, :, col0 : col0 + 4], glo[:], sg, tt, c)
                        emit_val(od4[:, :, :, col0 + 4 : col0 + 8], ghi[:], sg, tt, c)
                    else:
                        col0 = c * 8 if c < 7 else (c - 7) * 8 + 4
                        gv = gc[:].rearrange("n (p k c) -> n p k c", p=2, k=48)
                        emit_val(od4[:, :, :, col0 : col0 + 4], gv[:], sg, tt, c)
                nc.sync.dma_start(
                    out=out3[b0 : b0 + P, :, D_LO:D_HI],
                    in_=od[:].rearrange("n (p f) -> n p f", p=2),
                )
            # Deferred fills: both rings co-drain the write tail after the
            # last store is pushed.
            tail = regions[-N_TAIL_FILLS:]
            fill(nc.sync, tail[0::2])
            fill(nc.scalar, tail[1::2])
    nc.compile()
    return nc


def make_weights():
    w = np.tile((2.0 ** np.arange(NBITS)).astype(np.float32), NCODE)
    return np.broadcast_to(w, (P, NCODE * NBITS)).copy()


def make_tn(table):
    t = np.asarray(table).reshape(L, 2, 48, 8)
    tn = np.empty((L, ROW), dtype=np.float16)
    tn[:, :HROW] = t[:, :, :, 0:4].reshape(L, HROW)
    tn[:, HROW:] = t[:, :, :, 4:8].reshape(L, HROW)
    return tn


def make_in_maps(x, table):
    tn = make_tn(table)
    w = make_weights()
    return [
        {
            "x": np.ascontiguousarray(x[i * BC : (i + 1) * BC]),
            "table": tn,
            "w": w,
        }
        for i in range(NCORES)
    ]


_NC_CACHE = None


def _get_module():
    global _NC_CACHE
    if _NC_CACHE is None:
        _NC_CACHE = build_module()
    return _NC_CACHE


def kernel(x: np.ndarray, table: np.ndarray) -> np.ndarray:
    from concourse.bass_utils import run_bass_kernel_spmd

    x = np.asarray(x)
    table = np.asarray(table)
    assert x.shape == (BATCH, XCOLS) and table.shape == (L, 2, 48, 8)
    nc = _get_module()
    res = run_bass_kernel_spmd(nc, make_in_maps(x, table), core_ids=list(range(NCORES)))
    return np.concatenate([res.results[i]["out"] for i in range(NCORES)], axis=0)
